# revision 1
# baseline (speedup 1.0000x reference)
# Trainium2 Bass kernel for nn_Invo2D (involution-style dynamic conv).
#
# Math (verified vs reference, rel err ~4e-7):
#   t1 = x @ W1 + b1                      [pix, 64]
#   t2 = t1 @ W2 + b2                     [pix, 144] = [g:16, j:9]
#   P[pix, f] = 3x3 SAME patches, f = ki*256 + ch   (ki row-major tap)
#   out[pix, co] = sum_j t2[pix, 9*(co//16)+j] * P[pix, 9*co+j]
#
# Sharding: data-parallel over batch, 1 image per NeuronCore (8 cores).
#
# Per-core layout: partition p = wq*64 + h (wq = w//32), per-partition free dim
# holds the 32 pixels (wl = w%32) of that image row-half.  Spatial taps become
# free-dim offsets (w) plus partition-shifted copies of x (h +- 1), so the
# data-dependent multiply-reduce runs lane-local on the Vector engine.

import numpy as np
import ml_dtypes

H, W, C = 64, 64, 256
G, GC, KK = 16, 16, 9
M144, D = 144, 64
NCORES = 8
SLOTS = 34            # w slots per partition: slot s <-> w = 32*wq + s - 1
XF = SLOTS * C        # 8704 x-elems per partition
WLC = 8               # wl chunk size
NCHUNK = 32 // WLC    # 4 chunks
W16F = 32 * 256       # W16 free size (all 32 wl)
M16F = WLC * 4096     # product-chunk free size

_cache = {}


def _rect_decomp(r0, r1):
    """[r0, r1) in (gc, j) space (gc = r//9, j = r%9) -> rects (gc0, ngc, j0, nj)."""
    out = []
    gc0, j0 = divmod(r0, 9)
    if j0 != 0:
        end = min(r1, (gc0 + 1) * 9)
        out.append((gc0, 1, j0, end - r0))
        r0 = end
        if r0 == r1:
            return out
        gc0, j0 = divmod(r0, 9)
    nfull = (r1 - r0) // 9
    if nfull:
        out.append((gc0, nfull, 0, 9))
        r0 += nfull * 9
        gc0 += nfull
    if r0 < r1:
        out.append((gc0, 1, 0, r1 - r0))
    return out


def _build_pieces():
    """Mult pieces: (g, gc0, ngc, j0, nj, ki). Each is one contiguous f-run
    within one spatial tap ki, rectangular in (gc, j)."""
    pieces = []
    for g in range(16):
        f_lo, f_hi = 144 * g, 144 * g + 144
        cuts = [f_lo] + [256 * k for k in range(1, 9) if f_lo < 256 * k < f_hi] + [f_hi]
        for a, b in zip(cuts, cuts[1:]):
            ki = a // 256
            for (gc0, ngc, j0, nj) in _rect_decomp(a - 144 * g, b - 144 * g):
                pieces.append((g, gc0, ngc, j0, nj, ki))
    return pieces


def _build_program():
    import concourse.bass as bass
    import concourse.tile as tile
    from concourse import bacc, mybir
    from concourse.masks import make_identity

    f32 = mybir.dt.float32
    bf16 = mybir.dt.bfloat16
    AP = bass.AP

    nc = bacc.Bacc(None, target_bir_lowering=False)
    x_d = nc.dram_tensor("x", [H, W, C], f32, kind="ExternalInput")
    w1_d = nc.dram_tensor("w1", [2, 128, D], bf16, kind="ExternalInput")
    b1_d = nc.dram_tensor("b1row", [1, D], bf16, kind="ExternalInput")
    w2_d = nc.dram_tensor("w2aug", [D + 1, M144], bf16, kind="ExternalInput")
    out_d = nc.dram_tensor("out", [H, W, C], f32, kind="ExternalOutput")

    PIECES = _build_pieces()

    with tile.TileContext(nc) as tc:
        with (
            tc.tile_pool(name="singles", bufs=1) as singles,
            tc.tile_pool(name="xbufs", bufs=1) as xbufs,
            tc.tile_pool(name="big", bufs=1) as big,
            tc.tile_pool(name="folds", bufs=1) as folds,
            tc.tile_pool(name="t1p", bufs=3) as t1p,
            tc.tile_pool(name="pst", bufs=2, space="PSUM") as pst,
            tc.tile_pool(name="ps1", bufs=2, space="PSUM") as ps1p,
            tc.tile_pool(name="ps2", bufs=2, space="PSUM") as ps2p,
        ):
            # ---- constants ----
            ident = singles.tile([128, 128], bf16)
            make_identity(nc, ident[:])
            w1a = singles.tile([128, D], bf16)
            w1b = singles.tile([128, D], bf16)
            nc.sync.dma_start(out=w1a[:], in_=w1_d[0])
            nc.sync.dma_start(out=w1b[:], in_=w1_d[1])
            b1sb = singles.tile([1, D], bf16)
            nc.sync.dma_start(out=b1sb[:], in_=b1_d[:])
            w2sb = singles.tile([D + 1, M144], bf16)
            nc.sync.dma_start(out=w2sb[:], in_=w2_d[:])
            ones1 = singles.tile([1, 128], bf16)
            nc.gpsimd.memset(ones1[:], 1.0)

            # ---- x staging (fp32, h-major padded) + bf16 cast + shifted copies ----
            stage = big.tile([128, XF], f32, tag="bigslot")
            # wq=0: rows h -> parts 0..63, w 0..32 -> slots 1..33
            nc.sync.dma_start(
                out=AP(stage.tensor, 256, [[XF, 64], [1, 33 * 256]]),
                in_=AP(x_d, 0, [[W * C, 64], [1, 33 * 256]]),
            )
            # wq=1: rows h -> parts 64..127, w 31..63 -> slots 0..32
            nc.sync.dma_start(
                out=AP(stage.tensor, 64 * XF, [[XF, 64], [1, 33 * 256]]),
                in_=AP(x_d, 31 * 256, [[W * C, 64], [1, 33 * 256]]),
            )
            X0 = xbufs.tile([128, XF], bf16)
            XU = xbufs.tile([128, XF], bf16)   # row h+1
            XD = xbufs.tile([128, XF], bf16)   # row h-1
            nc.vector.tensor_copy(out=X0[:], in_=stage[:])
            # zero pads: wq0 slot0 (w=-1), wq1 slot33 (w=64)
            nc.gpsimd.memset(AP(X0.tensor, 0, [[XF, 64], [1, 256]]), 0.0)
            nc.gpsimd.memset(AP(X0.tensor, 64 * XF + 33 * 256, [[XF, 64], [1, 256]]), 0.0)
            # XU[p] = X0[p+1] within each half; edge rows zero
            nc.sync.dma_start(out=AP(XU.tensor, 0, [[XF, 63], [1, XF]]),
                              in_=AP(X0.tensor, XF, [[XF, 63], [1, XF]]))
            nc.sync.dma_start(out=AP(XU.tensor, 64 * XF, [[XF, 63], [1, XF]]),
                              in_=AP(X0.tensor, 65 * XF, [[XF, 63], [1, XF]]))
            # partition starts must be 32-aligned for engine ops; zero rows
            # 63/127 by DMA from a zeroed row instead
            zrow = singles.tile([1, XF], bf16)
            nc.gpsimd.memset(zrow[:], 0.0)
            nc.sync.dma_start(out=AP(XU.tensor, 63 * XF, [[XF, 1], [1, XF]]),
                              in_=zrow[:])
            nc.sync.dma_start(out=AP(XU.tensor, 127 * XF, [[XF, 1], [1, XF]]),
                              in_=zrow[:])
            nc.sync.dma_start(out=AP(XD.tensor, XF, [[XF, 63], [1, XF]]),
                              in_=AP(X0.tensor, 0, [[XF, 63], [1, XF]]))
            nc.sync.dma_start(out=AP(XD.tensor, 65 * XF, [[XF, 63], [1, XF]]),
                              in_=AP(X0.tensor, 64 * XF, [[XF, 63], [1, XF]]))
            nc.gpsimd.memset(AP(XD.tensor, 0, [[XF, 1], [1, XF]]), 0.0)
            nc.gpsimd.memset(AP(XD.tensor, 64 * XF, [[XF, 1], [1, XF]]), 0.0)

            # ---- x transposes -> channel-major xcm [ch(128), half(2), tile(32), col(128)] ----
            xcm = singles.tile([128, 2 * 32 * 128], bf16)
            for half in range(2):
                for grp in range(4):
                    pt = pst.tile([128, 1024], bf16)
                    for tt in range(8):
                        t = grp * 8 + tt
                        nc.tensor.transpose(
                            out=pt[:, tt * 128:(tt + 1) * 128],
                            in_=AP(X0.tensor, (t + 1) * 256 + half * 128,
                                   [[XF, 128], [1, 128]]),
                            identity=ident[:],
                        )
                    nc.scalar.copy(
                        out=xcm[:, (half * 32 + grp * 8) * 128:(half * 32 + grp * 8 + 8) * 128],
                        in_=pt[:],
                    )

            # ---- per-tile matmuls: t1 = x@W1+b1 (chan-major), t2 = t1aug@W2aug ----
            # tile t covers pixels w in {t, t+32}; psum partition p=(wq,h) matches
            # the h-major layout, so W16 fills lane-locally.
            W16c = [big.tile([128, WLC * 256], bf16, name=f"w16_{c}", tag=f"w16_{c}")
                    for c in range(NCHUNK)]
            for t in range(32):
                ps1 = ps1p.tile([D, 128], f32)
                xc0 = AP(xcm.tensor, (0 * 32 + t) * 128, [[2 * 32 * 128, 128], [1, 128]])
                xc1 = AP(xcm.tensor, (1 * 32 + t) * 128, [[2 * 32 * 128, 128], [1, 128]])
                nc.tensor.matmul(ps1[:], lhsT=w1a[:], rhs=xc0, start=True, stop=False)
                nc.tensor.matmul(ps1[:], lhsT=w1b[:], rhs=xc1, start=False, stop=False)
                nc.tensor.matmul(ps1[:], lhsT=b1sb[:], rhs=ones1[:],
                                 start=False, stop=True)
                t1t = t1p.tile([D + 1, 128], bf16)
                nc.scalar.copy(out=t1t[0:D, :], in_=ps1[:])
                nc.gpsimd.memset(t1t[D:D + 1, :], 1.0)
                ps2 = ps2p.tile([128, M144], f32)
                nc.tensor.matmul(ps2[:], lhsT=t1t[:], rhs=w2sb[:], start=True, stop=True)
                # scatter t2[m=9g+j] into 16-padded weight slots [wl=t%WLC, 16g+j]
                c, wl = divmod(t, WLC)
                nc.scalar.copy(
                    out=AP(W16c[c].tensor, wl * 256, [[WLC * 256, 128], [16, 16], [1, 9]]),
                    in_=AP(ps2.tensor, 0, [[M144, 128], [9, 16], [1, 9]]),
                )

            # ---- involution multiply (DVE) + fold + store, per wl-chunk ----
            M16 = big.tile([128, M16F], bf16, tag="bigslot")  # reuses stage slot
            F1 = folds.tile([128, WLC * 1024], bf16)
            F2 = folds.tile([128, WLC * 512], bf16)
            F3 = folds.tile([128, WLC * 256], bf16)
            outc = folds.tile([128, WLC * 256], f32)
            XBUF = {-1: XD, 0: X0, 1: XU}
            for c in range(NCHUNK):
                wl0 = c * WLC
                for (g, gc0, ngc, j0, nj, ki) in PIECES:
                    di, dj = ki // 3 - 1, ki % 3 - 1
                    xb = XBUF[di]
                    ch0 = 144 * g + 9 * gc0 + j0 - 256 * ki
                    in0 = AP(xb.tensor, (wl0 + dj + 1) * 256 + ch0,
                             [[XF, 128], [256, WLC], [9, ngc], [1, nj]])
                    in1 = AP(W16c[c].tensor, 16 * g + j0,
                             [[WLC * 256, 128], [256, WLC], [0, ngc], [1, nj]])
                    o = AP(M16.tensor, (16 * g + gc0) * 16 + j0,
                           [[M16F, 128], [4096, WLC], [16, ngc], [1, nj]])
                    nc.vector.tensor_mul(o, in0, in1)
                # fold 9 taps (jslots 0..8): (0..3)+(4..7), then halve, + slot 8
                nc.vector.tensor_add(
                    AP(F1.tensor, 0, [[WLC * 1024, 128], [1024, WLC], [4, 256], [1, 4]]),
                    AP(M16.tensor, 0, [[M16F, 128], [4096, WLC], [16, 256], [1, 4]]),
                    AP(M16.tensor, 4, [[M16F, 128], [4096, WLC], [16, 256], [1, 4]]),
                )
                nc.vector.tensor_add(
                    AP(F2.tensor, 0, [[WLC * 512, 128], [512, WLC], [2, 256], [1, 2]]),
                    AP(F1.tensor, 0, [[WLC * 1024, 128], [1024, WLC], [4, 256], [1, 2]]),
                    AP(F1.tensor, 2, [[WLC * 1024, 128], [1024, WLC], [4, 256], [1, 2]]),
                )
                nc.vector.tensor_add(
                    AP(F3.tensor, 0, [[WLC * 256, 128], [256, WLC], [1, 256]]),
                    AP(F2.tensor, 0, [[WLC * 512, 128], [512, WLC], [2, 256]]),
                    AP(F2.tensor, 1, [[WLC * 512, 128], [512, WLC], [2, 256]]),
                )
                nc.vector.tensor_add(
                    AP(outc.tensor, 0, [[WLC * 256, 128], [256, WLC], [1, 256]]),
                    AP(F3.tensor, 0, [[WLC * 256, 128], [256, WLC], [1, 256]]),
                    AP(M16.tensor, 8, [[M16F, 128], [4096, WLC], [16, 256]]),
                )
                nc.sync.dma_start(
                    out=AP(out_d, wl0 * 256, [[W * C, 64], [1, WLC * 256]]),
                    in_=AP(outc.tensor, 0, [[WLC * 256, 64], [1, WLC * 256]]),
                )
                nc.sync.dma_start(
                    out=AP(out_d, (32 + wl0) * 256, [[W * C, 64], [1, WLC * 256]]),
                    in_=AP(outc.tensor, 64 * WLC * 256, [[WLC * 256, 64], [1, WLC * 256]]),
                )
    nc.compile()
    return nc


def _get_program():
    if "nc" not in _cache:
        _cache["nc"] = _build_program()
    return _cache["nc"]


def kernel(x, W1, b1, W2, b2, trace=False):
    from concourse.bass_utils import run_bass_kernel_spmd

    nc = _get_program()
    bf = ml_dtypes.bfloat16
    w1_h = np.ascontiguousarray(W1.astype(bf).reshape(2, 128, D))
    b1_h = np.ascontiguousarray(b1.astype(bf).reshape(1, D))
    w2_h = np.ascontiguousarray(
        np.concatenate([W2, b2[None, :]], axis=0).astype(bf))
    in_maps = [
        {
            "x": np.ascontiguousarray(x[i], dtype=np.float32),
            "w1": w1_h,
            "b1row": b1_h,
            "w2aug": w2_h,
        }
        for i in range(NCORES)
    ]
    res = run_bass_kernel_spmd(nc, in_maps, core_ids=list(range(NCORES)),
                               trace=trace)
    out = np.stack([res.results[i]["out"] for i in range(NCORES)], axis=0)
    if trace:
        return out.astype(np.float32), res
    return out.astype(np.float32)

